# revision 18
# baseline (speedup 1.0000x reference)
"""Tensor-parallel GQA attention kernel for 8 Trainium2 NeuronCores.

Sharding: head-parallel. Core c computes q heads [4c, 4c+4) and kv head c
(GQA group), then the output projection is computed column-sharded after an
AllGather of the per-core context (split into two S-halves so the gather
overlaps attention and the output projection). Host concatenates the 8
output shards.

All matmuls run in float32r (full PE speed, ~TF32 precision).
"""

import math
import sys

import numpy as np

sys.path.insert(0, "/opt/trn_rl_repo")

# ---- problem constants (hardcoded per harness contract) ----
DIM = 4096
N_HEADS = 32
N_KV_HEADS = 8
HEAD_DIM = 128
N_REP = 4
SEQ = 2048
BATCH = 1
NCORES = 8

P = 128
KO = DIM // P        # 32 contraction chunks
SQ = 512             # seq tile width (matmul moving free dim)
NSQ = SEQ // SQ      # 4
NKS = SEQ // P       # 16 key tiles of 128
NH_LOC = N_HEADS // NCORES   # 4 local q heads
MQKV = NH_LOC * HEAD_DIM + 2 * HEAD_DIM  # 768 rows of fused qkv projection
DOUT = DIM // NCORES  # 512 output columns per core
SH = SEQ // 2         # 1024, AllGather half width
SCALE = 1.0 / math.sqrt(HEAD_DIM)

XB = 4               # k-chunks per xT load (512 KB DMAs)

_CACHE = {}


def _build():
    """Build and compile the Bass kernel once per process."""
    if "nc" in _CACHE:
        return _CACHE["nc"]

    import concourse.bacc as bacc
    import concourse.mybir as mybir
    import concourse.tile as tile
    from concourse.masks import make_identity
    from contextlib import ExitStack

    F32 = mybir.dt.float32
    F32R = mybir.dt.float32r
    MULT = mybir.AluOpType.mult
    ADD = mybir.AluOpType.add
    SUB = mybir.AluOpType.subtract
    EXP = mybir.ActivationFunctionType.Exp

    nc = bacc.Bacc(None, target_bir_lowering=False, debug=False)

    xT = nc.declare_dram_parameter("xt", [P, NSQ, KO, SQ], F32R, isOutput=False)
    wqkv = nc.declare_dram_parameter("wqkv", [P, KO, MQKV], F32R, isOutput=False)
    wo = nc.declare_dram_parameter("wo", [P, KO, DOUT], F32R, isOutput=False)
    cosd = nc.declare_dram_parameter("cost", [P, SEQ], F32, isOutput=False)
    sind = nc.declare_dram_parameter("sint", [P, SEQ], F32, isOutput=False)
    maskd = nc.declare_dram_parameter("masks", [P, 4, SQ], F32, isOutput=False)
    out = nc.declare_dram_parameter("o", [DOUT, SEQ], F32, isOutput=True)

    with tile.TileContext(nc) as tc, ExitStack() as stack:
        singles = stack.enter_context(tc.tile_pool(name="singles", bufs=1))
        dram = stack.enter_context(tc.tile_pool(name="dram", bufs=1, space="DRAM"))

        # asymmetric S-split for the staged AllGather: [0:1024], [1024:1536],
        # [1536:2048] — the big chunk gathers early under attention, the small
        # tail chunks keep the post-attention exposure short.
        CCW = (2 * SQ, SQ, SQ)
        cc_in = [dram.tile([NH_LOC * HEAD_DIM, wdt], F32, name=f"ccin{i}")
                 for i, wdt in enumerate(CCW)]
        cc_out = [dram.tile([N_HEADS * HEAD_DIM, wdt], F32, addr_space="Shared",
                            name=f"ccout{i}") for i, wdt in enumerate(CCW)]

        cos_sb = singles.tile([P, SEQ], F32)
        sin_sb = singles.tile([P, SEQ], F32)
        nc.sync.dma_start(cos_sb[:], cosd[:])
        nc.sync.dma_start(sin_sb[:], sind[:])

        idn = singles.tile([P, P], F32)
        make_identity(nc, idn)

        ones_f = singles.tile([P, 1], F32)
        nc.vector.memset(ones_f[:], 1.0)
        ones_col = singles.tile([P, 1], F32R)
        nc.vector.tensor_copy(ones_col[:], ones_f[:])
        ones_row_f = singles.tile([1, P], F32)
        nc.vector.memset(ones_row_f[:], 1.0)
        ones_row = singles.tile([1, P], F32R)
        nc.vector.tensor_copy(ones_row[:], ones_row_f[:])

        # attention operands, resident across phases 1-2
        qsb = singles.tile([P, NH_LOC, SEQ], F32R)   # per head: rows 0:64 re, 64:128 im
        kTsb = singles.tile([P, SEQ], F32R)
        vTsb = singles.tile([P, SEQ], F32)
        vsb = singles.tile([P, NKS, HEAD_DIM], F32R)

        # ---------------- Phase 1: fused QKV projection + RoPE ----------------
        # m-tile order chosen so PSUM tiles are revisited in the order the
        # RoPE eviction frees them (pairs (0,3), (1,4), (2,5)).
        M_ORDER = (0, 3, 1, 4, 2, 5)
        with tc.tile_pool(name="wq", bufs=1) as wpool, \
             tc.tile_pool(name="xtp", bufs=2) as xpool, \
             tc.tile_pool(name="rt", bufs=2) as rpool, \
             tc.tile_pool(name="ps1", bufs=1, space="PSUM") as pp1:
            w = []
            for g in range(KO // 4):
                wg = wpool.tile([P, 4, MQKV], F32R, tag=f"w{g}", name=f"w{g}")
                nc.sync.dma_start(wg[:], wqkv[:, 4 * g:4 * g + 4, :])
                w.append(wg)

            def wslice(k, m):
                return w[k // 4][:, k % 4, m * P:(m + 1) * P]

            for sq in range(NSQ):
                cols = slice(sq * SQ, (sq + 1) * SQ)
                pq = [pp1.tile([P, SQ], F32, tag=f"p{m}", name=f"p{m}_{sq}")
                      for m in range(6)]
                for xb in range(KO // XB):
                    xk = xpool.tile([P, XB, SQ], F32R, tag="xt", name=f"x{sq}_{xb}")
                    nc.sync.dma_start(xk[:], xT[:, sq, xb * XB:(xb + 1) * XB, :])
                    for kk in range(XB):
                        k = xb * XB + kk
                        for m in M_ORDER:
                            nc.tensor.matmul(pq[m][:], wslice(k, m), xk[:, kk, :],
                                             start=(k == 0), stop=(k == KO - 1))

                # RoPE eviction. m-tile pairs: (0,3)->(q0,q1), (1,4)->(q2,q3),
                # (2,5)->(k | v-halves). Full-width multiplies first (frees the
                # PSUM pair after 4 ops), then 64-row combines into the heads.
                for i, (h0, h1) in enumerate(((0, 1), (2, 3), (4, 5))):
                    A, B = pq[i][:], pq[i + 3][:]
                    tac = rpool.tile([P, SQ], F32, tag="tac")   # A*cos
                    tas = rpool.tile([P, SQ], F32, tag="tas")   # A*sin
                    tbs = rpool.tile([P, SQ], F32, tag="tbs")   # B*sin
                    tbc = rpool.tile([P, SQ], F32, tag="tbc")   # B*cos
                    nc.vector.tensor_tensor(tac[:], A, cos_sb[:, cols], MULT)
                    nc.vector.tensor_tensor(tas[:], A, sin_sb[:, cols], MULT)
                    if i == 2:
                        # v passthrough straight from PSUM (frees pq[2]/pq[5])
                        nc.vector.tensor_copy(vTsb[0:64, cols], A[64:128])
                    nc.vector.tensor_tensor(tbs[:], B, sin_sb[:, cols], MULT)
                    nc.vector.tensor_tensor(tbc[:], B, cos_sb[:, cols], MULT)
                    if i == 2:
                        nc.vector.tensor_copy(vTsb[64:128, cols], B[64:128])
                        dests = ((slice(0, 64), kTsb[0:64, cols],
                                  kTsb[64:128, cols]),)
                    else:
                        h0q, h1q = 2 * i, 2 * i + 1
                        dests = ((slice(0, 64), qsb[0:64, h0q, cols],
                                  qsb[64:128, h0q, cols]),
                                 (slice(64, 128), qsb[0:64, h1q, cols],
                                  qsb[64:128, h1q, cols]))
                    for half, dre, dim_ in dests:
                        nc.vector.tensor_tensor(dre, tac[half], tbs[half], SUB)
                        nc.vector.tensor_tensor(dim_, tas[half], tbc[half], ADD)

                # transpose this quarter's v chunks: vT [128, s] -> v [s, 128]
                for t in range(4 * sq, 4 * sq + 4):
                    ptr = pp1.tile([P, P], F32, tag="ptr", bufs=2, name=f"ptr{t}")
                    nc.tensor.transpose(ptr[:], vTsb[:, t * P:(t + 1) * P], idn[:])
                    nc.scalar.copy(vsb[:, t, :], ptr[:])

        # masks first (small, needed at the first attention tile), then wo
        mpool0 = stack.enter_context(tc.tile_pool(name="mp", bufs=1))
        mask_sb = mpool0.tile([P, 4, SQ], F32)
        nc.sync.dma_start(mask_sb[:], maskd[:])
        wopool = stack.enter_context(tc.tile_pool(name="wopool", bufs=1))
        wo_sb = wopool.tile([P, KO, DOUT], F32R)
        nc.sync.dma_start(wo_sb[:], wo[:])

        # ---------------- Phase 2: causal GQA attention ----------------
        with tc.tile_pool(name="pt", bufs=4) as ptpool, \
             tc.tile_pool(name="st", bufs=2) as stpool, \
             tc.tile_pool(name="ps2", bufs=1, space="PSUM") as pp2:
            DIV = mybir.AluOpType.divide
            pending_fin = [None]

            def emit_fin():
                if pending_fin[0] is not None:
                    pending_fin[0]()
                    pending_fin[0] = None

            for j in range(NSQ):
                for h in range(NH_LOC):
                    nks = 4 * (j + 1)
                    qcols = slice(j * SQ, (j + 1) * SQ)
                    ps_ctx = pp2.tile([P, SQ], F32, tag="ctx", bufs=2,
                                      name=f"ctx{h}_{j}")
                    ps_den = pp2.tile([1, SQ], F32, tag="den", bufs=2,
                                      name=f"den{h}_{j}")

                    # software pipeline: scores/exp run 2 tiles ahead of PV/den
                    def do_scores(t, h=h, j=j, qcols=qcols):
                        ps_s = pp2.tile([P, SQ], F32, tag="s", bufs=3,
                                        name=f"s{h}_{j}_{t}")
                        nc.tensor.matmul(ps_s[:], kTsb[:, t * P:(t + 1) * P],
                                         qsb[:, h, qcols], start=True, stop=True)
                        pT = ptpool.tile([P, SQ], F32R, tag="pT",
                                         name=f"pT{h}_{j}_{t}")
                        nc.scalar.activation(pT[:], ps_s[:], EXP, scale=SCALE)
                        if t >= 4 * j:
                            nc.vector.tensor_tensor(pT[:], pT[:].bitcast(F32),
                                                    mask_sb[:, t - 4 * j, :], MULT)
                        return pT

                    def do_pv(t, pT, ps_ctx=ps_ctx, ps_den=ps_den, nks=nks):
                        nc.tensor.matmul(ps_ctx[:], vsb[:, t, :], pT[:],
                                         start=(t == 0), stop=(t == nks - 1))
                        nc.tensor.matmul(ps_den[0:1, :], ones_col[:], pT[:],
                                         start=(t == 0), stop=(t == nks - 1))

                    pend = {}
                    for t in range(nks):
                        pend[t] = do_scores(t)
                        if t == 3:
                            # previous (h, j)'s epilogue, deferred so its PE
                            # matmul never stalls the stream
                            emit_fin()
                        if t >= 2:
                            do_pv(t - 2, pend.pop(t - 2))
                    emit_fin()
                    for t in (nks - 2, nks - 1):
                        do_pv(t, pend.pop(t))

                    # denominator row to SBUF now (ACT), rest deferred
                    den_sb = stpool.tile([1, SQ], F32R, tag="den_sb")
                    nc.scalar.copy(den_sb[:], ps_den[0:1, :])

                    def fin(h=h, j=j, ps_ctx=ps_ctx, den_sb=den_sb):
                        ps_bc = pp2.tile([P, SQ], F32, tag="bc", bufs=1,
                                         name=f"bc{h}_{j}")
                        nc.tensor.matmul(ps_bc[:], ones_row[:], den_sb[:],
                                         start=True, stop=True)
                        bc_sb = stpool.tile([P, SQ], F32, tag="bc_sb")
                        nc.vector.tensor_copy(bc_sb[:], ps_bc[:])
                        rc_sb = stpool.tile([P, SQ], F32, tag="rc_sb")
                        nc.vector.reciprocal(rc_sb[:], bc_sb[:])
                        ctx_st = stpool.tile([P, SQ], F32, tag="ctx_st")
                        nc.vector.tensor_tensor(ctx_st[:], ps_ctx[:], rc_sb[:], MULT)
                        cci = 0 if j < 2 else j - 1
                        co = (j % 2) * SQ if j < 2 else 0
                        nc.sync.dma_start(
                            cc_in[cci].rearrange("(h p) s -> p h s", p=P)
                                      [:, h, co:co + SQ],
                            ctx_st[:])

                    pending_fin[0] = fin

                if j >= 1:
                    emit_fin()
                    nc.gpsimd.collective_compute(
                        "AllGather", mybir.AluOpType.bypass,
                        replica_groups=[list(range(NCORES))],
                        ins=[cc_in[j - 1][:]], outs=[cc_out[j - 1][:]])

        # ---------------- Phase 3: output projection (column shard) ----------------
        with tc.tile_pool(name="cx", bufs=4) as cxpool, \
             tc.tile_pool(name="os", bufs=2) as ospool, \
             tc.tile_pool(name="ps3", bufs=1, space="PSUM") as pp3:
            for part in range(3):
                wdt = CCW[part]
                nsub = wdt // SQ
                soff = 0 if part == 0 else (part + 1) * SQ
                ccv = cc_out[part].rearrange("(ko p) s -> p ko s", p=P)
                po = [pp3.tile([P, SQ], F32, tag=f"o{i}", bufs=1,
                               name=f"o{i}_{part}") for i in range(4 * nsub)]
                for ko in range(KO):
                    cxt = cxpool.tile([P, wdt], F32R, tag="cx",
                                      name=f"cx{part}_{ko}")
                    nc.sync.dma_start(cxt[:], ccv[:, ko, :].bitcast(F32R))
                    for dt in range(4):
                        for sub in range(nsub):
                            nc.tensor.matmul(
                                po[dt * nsub + sub][:],
                                wo_sb[:, ko, dt * P:(dt + 1) * P],
                                cxt[:, sub * SQ:(sub + 1) * SQ],
                                start=(ko == 0), stop=(ko == KO - 1))
                out_sb = ospool.tile([P, 4, wdt], F32, tag="osb",
                                     name=f"osb{part}")
                for dt in range(4):
                    for sub in range(nsub):
                        nc.scalar.copy(out_sb[:, dt, sub * SQ:(sub + 1) * SQ],
                                       po[dt * nsub + sub][:])
                nc.sync.dma_start(
                    out.rearrange("(dt p) s -> p dt s", p=P)
                       [:, :, soff:soff + wdt],
                    out_sb[:])

    nc.compile()
    _CACHE["nc"] = nc
    return nc


def _prep_inputs(x, wq, wk, wv, wo, freqs_cos, freqs_sin):
    """Host-side sharding + layout prep. Returns in_maps for the 8 cores."""
    x = np.asarray(x, dtype=np.float32)
    wq = np.asarray(wq, dtype=np.float32)
    wk = np.asarray(wk, dtype=np.float32)
    wv = np.asarray(wv, dtype=np.float32)
    wo = np.asarray(wo, dtype=np.float32)
    freqs_cos = np.asarray(freqs_cos, dtype=np.float32)
    freqs_sin = np.asarray(freqs_sin, dtype=np.float32)

    # xT in [P, NSQ, KO, SQ] layout: element (d, s), d = ko*128 + p, s = sq*SQ + s'
    xT = np.ascontiguousarray(
        x[0].T.reshape(KO, P, NSQ, SQ).transpose(1, 2, 0, 3))

    # rotate-half permutation within a head: [0,2,4,...126, 1,3,...,127]
    perm = np.concatenate([np.arange(0, HEAD_DIM, 2), np.arange(1, HEAD_DIM, 2)])

    # cos/sin tables transposed and duplicated across both 64-row halves
    cosT = np.ascontiguousarray(freqs_cos.T)  # [64, SEQ]
    sinT = np.ascontiguousarray(freqs_sin.T)
    cos2 = np.concatenate([cosT, cosT], axis=0)  # [128, SEQ]
    sin2 = np.concatenate([sinT, sinT], axis=0)

    # causal mask tiles: mask_r[i, jl] = 1 if jl - i >= 128*r
    i_idx = np.arange(P)[:, None]
    j_idx = np.arange(SQ)[None, :]
    masks = np.stack([(j_idx - i_idx >= P * r).astype(np.float32) for r in range(4)],
                     axis=0)  # [4, 128, SQ]
    masks_l = np.ascontiguousarray(masks.transpose(1, 0, 2))  # [P, 4, SQ]

    in_maps = []
    for c in range(NCORES):
        # fused qkv weight rows, permuted for RoPE (re/im separated by m-tile)
        qh = [wq[(4 * c + h) * HEAD_DIM:(4 * c + h + 1) * HEAD_DIM][perm]
              for h in range(NH_LOC)]  # each [128, DIM], rows [re(64); im(64)]
        kh = wk[c * HEAD_DIM:(c + 1) * HEAD_DIM][perm]  # [128, DIM]
        vh = wv[c * HEAD_DIM:(c + 1) * HEAD_DIM]        # [128, DIM] original order
        rows = np.empty((MQKV, DIM), dtype=np.float32)
        rows[0:64] = qh[0][0:64]        # tile0: q0 re | q1 re
        rows[64:128] = qh[1][0:64]
        rows[128:192] = qh[2][0:64]     # tile1: q2 re | q3 re
        rows[192:256] = qh[3][0:64]
        rows[256:320] = kh[0:64]        # tile2: k re | v dims 0:64
        rows[320:384] = vh[0:64]
        rows[384:448] = qh[0][64:128]   # tile3: q0 im | q1 im
        rows[448:512] = qh[1][64:128]
        rows[512:576] = qh[2][64:128]   # tile4: q2 im | q3 im
        rows[576:640] = qh[3][64:128]
        rows[640:704] = kh[64:128]      # tile5: k im | v dims 64:128
        rows[704:768] = vh[64:128]
        wqkvT = np.ascontiguousarray(
            rows.T.reshape(KO, P, MQKV).transpose(1, 0, 2))  # [P, KO, MQKV]

        woT = np.ascontiguousarray(
            wo[c * DOUT:(c + 1) * DOUT].T.reshape(KO, P, DOUT).transpose(1, 0, 2))

        in_maps.append({
            "xt": xT,
            "wqkv": wqkvT,
            "wo": woT,
            "cost": cos2,
            "sint": sin2,
            "masks": masks_l,
        })
    return in_maps


def run(inputs, trace=False, tmpdir=None):
    """Compile (cached), run on 8 cores, return (output, BassKernelResults)."""
    from concourse.bass_utils import run_bass_kernel_spmd

    nc = _build()
    in_maps = _prep_inputs(**inputs)
    res = run_bass_kernel_spmd(nc, in_maps, list(range(NCORES)),
                               trace=trace, tmpdir=tmpdir)
    out = np.empty((BATCH, SEQ, DIM), dtype=np.float32)
    for c in range(NCORES):
        out[0, :, c * DOUT:(c + 1) * DOUT] = res.results[c]["o"].T
    return out, res


def kernel(**inputs) -> np.ndarray:
    out, _ = run(inputs)
    return out
